# revision 28
# baseline (speedup 1.0000x reference)
"""Trainium2 Bass kernel for nn_EnergyBalanceChecker (segment_reduce).

Problem (hardcoded): B=4, N=512, T=24, G=32, TOL=0.05, EPS=1e-6.

  onehot[g,n] = (lv_group_ids[n] == g);  M = onehot * valid_lv_mask
  gc  = einsum('gn,bnt->bgt', M, consumption)
  gg  = einsum('gn,bnt->bgt', M, generation)
  net = einsum('gn,bnt->bgt', M, S.sum(axis=2) - S.sum(axis=1))
  pen = relu(|gc-gg+net| / (gc+gg+eps) - TOL);  out = pen.sum()*w/n_unique

Sharding: 8 cores = 4 batches x 2 halves of the (row) N axis. Each core
reads the contiguous block S[b, h*256:(h+1)*256, :, :] (12.6 MB) once and
emits per-group partials [3, 32, 24] = (pgc, pgg, pnet). The single PE
pass over the data computes both reductions at once: lhsT = [M^T_loc|ones]
gives PSUM rows 0..31 = M-projected rows (still per-(m,t)) and row 32 =
plain column sums. ACT drains PSUM to a [33, N*T] accumulator; DVE then
reduces rows 0..31 over m (row-sum term) while row 32 is regathered to
m-partitions and folded in with -M^T_full weights. Host sums the two
half partials per batch and applies the tiny nonlinear tail (~20 KFLOP).
"""

import sys

import numpy as np

try:
    import concourse  # noqa: F401
except ImportError:
    sys.path.insert(0, "/opt/trn_rl_repo")

import concourse.tile as tile
from concourse import bacc, mybir
from concourse.bass_utils import run_bass_kernel_spmd

B, N, T, G = 4, 512, 24, 32
TOL, EPS = 0.05, 1e-6
P = 128                 # SBUF partitions
NLOC = N // 2           # rows per core (n-half)
NB = NLOC // P          # 2 n-blocks of 128 rows
QM = 64                 # m-columns per streamed S tile
MB = N // QM            # number of (nb-pair) S tiles per core
FREE = QM * T           # free elements per (nb, mb) tile
MMCH = 512              # matmul free-dim chunk
EV = 768                # PSUM evacuation chunk (2 banks, 32 m-columns)
NEV = N * T // EV       # total evacuation chunks (16)
EVM = EV // T           # m-columns per evacuation chunk (32)
CT = N // P             # colT regather chunks of 128 m (4)

_F32 = mybir.dt.float32
_F16 = mybir.dt.float16


def _build_program():
    nc = bacc.Bacc("TRN2", target_bir_lowering=False, debug=False,
                   enable_asserts=False, num_devices=8)
    s = nc.dram_tensor("s", [NLOC, N, T], _F32, kind="ExternalInput").ap()
    cons = nc.dram_tensor("cons", [NLOC, T], _F32, kind="ExternalInput").ap()
    gen = nc.dram_tensor("gen", [NLOC, T], _F32, kind="ExternalInput").ap()
    mt_loc = nc.dram_tensor("mt_loc", [NLOC, G], _F32, kind="ExternalInput").ap()
    mt_neg = nc.dram_tensor("mt_neg", [N, G], _F32, kind="ExternalInput").ap()
    out = nc.dram_tensor("out", [3, G, T], _F32, kind="ExternalOutput").ap()

    with tile.TileContext(nc) as tc:
        with (
            tc.tile_pool(name="spool", bufs=NB * MB) as spool,
            tc.tile_pool(name="small", bufs=1) as small,
            tc.tile_pool(name="pcol", bufs=2, space="PSUM") as pcol,
            tc.tile_pool(name="pproj", bufs=1, space="PSUM") as pproj,
        ):
            # lhsT for the main pass: columns 0..31 = M^T rows for this
            # n-block, column 32 = ones (plain column sum). fp16, like the
            # streamed S tiles, for full-rate PE; PSUM accumulates fp32.
            lhsT = small.tile([P, NB, G + 1], _F16, tag="lhsT")
            nc.gpsimd.dma_start(out=lhsT[:, :, 0:G],
                                in_=mt_loc.rearrange("(nb p) g -> p nb g", p=P))
            nc.vector.memset(lhsT[:, :, G:G + 1], 1.0)
            lhsT32 = small.tile([P, NB, G], _F32, tag="lhsT32")
            nc.scalar.dma_start(out=lhsT32,
                                in_=mt_loc.rearrange("(nb p) g -> p nb g", p=P))
            mtn = small.tile([P, CT, G], _F32, tag="mtn")
            nc.scalar.dma_start(out=mtn, in_=mt_neg.rearrange("(mc p) g -> p mc g", p=P))
            cg = small.tile([P, 2, NB, T], _F32, tag="cg")
            nc.scalar.dma_start(out=cg[:, 0], in_=cons.rearrange("(nb p) t -> p nb t", p=P))
            nc.scalar.dma_start(out=cg[:, 1], in_=gen.rearrange("(nb p) t -> p nb t", p=P))

            colacc = small.tile([1, N * T], _F32, tag="colacc")
            colT = small.tile([P, CT, T], _F32, tag="colT")
            rowacc = small.tile([G, NEV, T], _F32, tag="rowacc")
            rowsum = small.tile([G, T], _F32, tag="rowsum")
            out_sb = small.tile([G, 3, T], _F32, tag="out_sb")

            pgc = pproj.tile([G, T], _F32, tag="pgc")
            pgg = pproj.tile([G, T], _F32, tag="pgg")
            pcp = pproj.tile([G, T], _F32, tag="pcp")

            # pgc / pgg: tiny projections of consumption / generation.
            for nb in range(NB):
                nc.tensor.matmul(pgc, lhsT32[:, nb], cg[:, 0, nb],
                                 start=(nb == 0), stop=(nb == NB - 1))
                nc.tensor.matmul(pgg, lhsT32[:, nb], cg[:, 1, nb],
                                 start=(nb == 0), stop=(nb == NB - 1))
            nc.scalar.copy(out=out_sb[:, 0], in_=pgc)
            nc.scalar.copy(out=out_sb[:, 1], in_=pgg)
            nc.scalar.dma_start(out=out[0:2].rearrange("k g t -> g k t"),
                                in_=out_sb[:, 0:2])

            # Stream all of S up front on the SP HWDGE ring.
            s4 = s.rearrange("(nb p) (mb q) t -> mb nb p (q t)", p=P, q=QM)
            stiles = {}
            for mb in range(MB):
                for nb in range(NB):
                    st = spool.tile([P, FREE], _F16, tag="s")
                    nc.gpsimd.dma_start(out=st, in_=s4[mb, nb])
                    stiles[(mb, nb)] = st

            # Flat loop over the 16 evacuation chunks (32 m-columns each).
            # nb outer within each PSUM tile so the stationary weights
            # reload NB times per tile, not per matmul.
            for q in range(NEV):
                pos = q * EV                    # global (m, t) flat offset
                mb, off = divmod(pos, FREE)     # source S tile and offset
                pc = pcol.tile([G + 1, EV], _F32, tag="pc")
                for nb in range(NB):
                    for c0 in range(0, EV, MMCH):
                        cw = min(MMCH, EV - c0)
                        nc.tensor.matmul(
                            pc[:, c0:c0 + cw],
                            lhsT[:, nb],
                            stiles[(mb, nb)][:, off + c0:off + c0 + cw],
                            start=(nb == 0), stop=(nb == NB - 1),
                            skip_group_check=True)
                # Two independent readers drain PSUM concurrently: ACT
                # keeps only the column-sum row, DVE folds the projected
                # rows over this chunk's 32 m-columns.
                nc.scalar.copy(out=colacc[:, pos:pos + EV],
                               in_=pc[G:G + 1, :])
                nc.vector.reduce_sum(
                    out=rowacc[:, q],
                    in_=pc[0:G, :].rearrange("p (m t) -> p t m", t=T),
                    axis=mybir.AxisListType.X,
                )
                # At each 128-m boundary: column sums to m-partitions
                # (ACT HWDGE ring, tiny) and fold -M^T @ colsum in.
                if (q + 1) % (P // EVM) == 0:
                    ct = q // (P // EVM)
                    nc.scalar.dma_start(
                        out=colT[:, ct],
                        in_=colacc[0:1, ct * P * T:(ct + 1) * P * T].rearrange(
                            "p (m t) -> p m t", t=T))
                    nc.tensor.matmul(pcp, mtn[:, ct], colT[:, ct],
                                     start=(ct == 0), stop=(ct == CT - 1))

            nc.vector.reduce_sum(
                out=rowsum, in_=rowacc[:].rearrange("p e t -> p t e"),
                axis=mybir.AxisListType.X)
            nc.vector.tensor_add(out_sb[:, 2], rowsum, pcp)
            nc.scalar.dma_start(out=out[2], in_=out_sb[:, 2])
    nc.compile()
    return nc


_NC_CACHE = None


def _get_program():
    global _NC_CACHE
    if _NC_CACHE is None:
        _NC_CACHE = _build_program()
    return _NC_CACHE


_RUNNER_CACHE = None


def _get_runner():
    """Compiled-once jit(shard_map) executor over 8 cores.

    Mirrors concourse.bass2jax.run_bass_via_pjrt but caches the traced
    function so repeat calls skip retracing/compile-cache lookups."""
    global _RUNNER_CACHE
    if _RUNNER_CACHE is None:
        import jax
        from jax.sharding import Mesh, PartitionSpec
        from jax.experimental.shard_map import shard_map
        from concourse import bass2jax, mybir as mb

        nc = _get_program()
        bass2jax.install_neuronx_cc_hook()
        partition_name = (nc.partition_id_tensor.name
                          if nc.partition_id_tensor else None)
        in_names, out_names, out_avals = [], [], []
        for alloc in nc.m.functions[0].allocations:
            if not isinstance(alloc, mb.MemoryLocationSet):
                continue
            name = alloc.memorylocations[0].name
            if alloc.kind == "ExternalInput":
                if name != partition_name:
                    in_names.append(name)
            elif alloc.kind == "ExternalOutput":
                out_names.append(name)
                out_avals.append(jax.core.ShapedArray(
                    tuple(alloc.tensor_shape), mb.dt.np(alloc.dtype)))
        n_params = len(in_names)
        all_names = in_names + out_names
        if partition_name is not None:
            all_names = all_names + [partition_name]

        def _body(*args):
            operands = list(args)
            if partition_name is not None:
                operands.append(bass2jax.partition_id_tensor())
            outs = bass2jax._bass_exec_p.bind(
                *operands,
                out_avals=tuple(out_avals),
                in_names=tuple(all_names),
                out_names=tuple(out_names),
                lowering_input_output_aliases=(),
                sim_require_finite=True,
                sim_require_nnan=True,
                nc=nc,
            )
            return tuple(outs)

        devices = jax.devices()[:8]
        mesh = Mesh(np.asarray(devices), ("core",))
        n_outs = len(out_names)
        sharded = jax.jit(
            shard_map(_body, mesh=mesh,
                      in_specs=(PartitionSpec("core"),) * (n_params + n_outs),
                      out_specs=(PartitionSpec("core"),) * n_outs,
                      check_rep=False),
            donate_argnums=tuple(range(n_params, n_params + n_outs)),
            keep_unused=True,
        )
        _RUNNER_CACHE = (sharded, in_names[:n_params], out_names, out_avals)
    return _RUNNER_CACHE


def kernel(consumption, generation, sharing_matrix, lv_group_ids,
           valid_lv_mask, imbalance_penalty_weight, _want_results=False,
           **run_kwargs):
    consumption = np.ascontiguousarray(consumption, dtype=np.float32)
    generation = np.ascontiguousarray(generation, dtype=np.float32)
    sharing_matrix = np.ascontiguousarray(sharing_matrix, dtype=np.float32)
    ids = np.asarray(lv_group_ids)
    valid = np.asarray(valid_lv_mask, dtype=np.float32)
    w = np.float32(np.asarray(imbalance_penalty_weight))

    onehot = (ids[None, :] == np.arange(G)[:, None]).astype(np.float32)
    n_unique = np.float32(np.unique(ids).size)
    M = onehot * valid[None, :]                      # [G, N]
    mt = np.ascontiguousarray(M.T)                   # [N, G]
    mt_neg = np.ascontiguousarray(-mt)

    in_maps = []
    for c in range(8):
        b, h = divmod(c, 2)
        sl = slice(h * NLOC, (h + 1) * NLOC)
        in_maps.append({
            "s": np.ascontiguousarray(sharing_matrix[b, sl]),
            "cons": np.ascontiguousarray(consumption[b, sl]),
            "gen": np.ascontiguousarray(generation[b, sl]),
            "mt_loc": np.ascontiguousarray(mt[sl]),
            "mt_neg": mt_neg,
        })
    res = None
    if _want_results or run_kwargs:
        nc = _get_program()
        res = run_bass_kernel_spmd(nc, in_maps, core_ids=list(range(8)),
                                   **run_kwargs)
        parts = np.stack([res.results[c]["out"] for c in range(8)])
    else:
        try:
            fn, in_names, out_names, out_avals = _get_runner()
            concat_in = [np.concatenate([m[name] for m in in_maps], axis=0)
                         for name in in_names]
            zeros = [np.zeros((8 * a.shape[0], *a.shape[1:]), a.dtype)
                     for a in out_avals]
            out_arrs = fn(*concat_in, *zeros)
            parts = np.asarray(out_arrs[out_names.index("out")]).reshape(
                8, 3, G, T)
        except Exception:
            nc = _get_program()
            res = run_bass_kernel_spmd(nc, in_maps, core_ids=list(range(8)))
            parts = np.stack([res.results[c]["out"] for c in range(8)])
    full = parts.reshape(B, 2, 3, G, T).sum(axis=1, dtype=np.float32)
    gc, gg, net = full[:, 0], full[:, 1], full[:, 2]

    imbalance = np.abs(gc - gg + net)
    total = gc + gg + np.float32(EPS)
    pen = np.maximum(imbalance / total - np.float32(TOL), np.float32(0))
    outv = np.float32(pen.sum(dtype=np.float32) * w / n_unique)
    out_arr = np.array(outv, dtype=np.float32)
    if _want_results:
        return out_arr, res
    return out_arr


# revision 29
# speedup vs baseline: 1.0521x; 1.0521x over previous
"""Trainium2 Bass kernel for nn_EnergyBalanceChecker (segment_reduce).

Problem (hardcoded): B=4, N=512, T=24, G=32, TOL=0.05, EPS=1e-6.

  onehot[g,n] = (lv_group_ids[n] == g);  M = onehot * valid_lv_mask
  gc  = einsum('gn,bnt->bgt', M, consumption)
  gg  = einsum('gn,bnt->bgt', M, generation)
  net = einsum('gn,bnt->bgt', M, S.sum(axis=2) - S.sum(axis=1))
  pen = relu(|gc-gg+net| / (gc+gg+eps) - TOL);  out = pen.sum()*w/n_unique

Sharding: 8 cores = 4 batches x 2 halves of the (row) N axis. Each core
reads the contiguous block S[b, h*256:(h+1)*256, :, :] (12.6 MB) once and
emits per-group partials [3, 32, 24] = (pgc, pgg, pnet). The single PE
pass over the data computes both reductions at once: lhsT = [M^T_loc|ones]
gives PSUM rows 0..31 = M-projected rows (still per-(m,t)) and row 32 =
plain column sums. ACT drains PSUM to a [33, N*T] accumulator; DVE then
reduces rows 0..31 over m (row-sum term) while row 32 is regathered to
m-partitions and folded in with -M^T_full weights. Host sums the two
half partials per batch and applies the tiny nonlinear tail (~20 KFLOP).
"""

import sys

import numpy as np

try:
    import concourse  # noqa: F401
except ImportError:
    sys.path.insert(0, "/opt/trn_rl_repo")

import concourse.tile as tile
from concourse import bacc, mybir
from concourse.bass_utils import run_bass_kernel_spmd

B, N, T, G = 4, 512, 24, 32
TOL, EPS = 0.05, 1e-6
P = 128                 # SBUF partitions
NLOC = N // 2           # rows per core (n-half)
NB = NLOC // P          # 2 n-blocks of 128 rows
QM = 64                 # m-columns per streamed S tile
MB = N // QM            # number of (nb-pair) S tiles per core
FREE = QM * T           # free elements per (nb, mb) tile
MMCH = 512              # matmul free-dim chunk
EV = 768                # PSUM evacuation chunk (2 banks, 32 m-columns)
NEV = N * T // EV       # total evacuation chunks (16)
EVM = EV // T           # m-columns per evacuation chunk (32)
CT = N // P             # colT regather chunks of 128 m (4)

_F32 = mybir.dt.float32
_F16 = mybir.dt.float16


def _build_program():
    nc = bacc.Bacc("TRN2", target_bir_lowering=False, debug=False,
                   enable_asserts=False, num_devices=8)
    s = nc.dram_tensor("s", [NLOC, N, T], _F32, kind="ExternalInput").ap()
    cons = nc.dram_tensor("cons", [NLOC, T], _F32, kind="ExternalInput").ap()
    gen = nc.dram_tensor("gen", [NLOC, T], _F32, kind="ExternalInput").ap()
    mt_loc = nc.dram_tensor("mt_loc", [NLOC, G], _F32, kind="ExternalInput").ap()
    mt_neg = nc.dram_tensor("mt_neg", [N, G], _F32, kind="ExternalInput").ap()
    out = nc.dram_tensor("out", [3, G, T], _F32, kind="ExternalOutput").ap()

    with tile.TileContext(nc) as tc:
        with (
            tc.tile_pool(name="spool", bufs=NB * MB) as spool,
            tc.tile_pool(name="small", bufs=1) as small,
            tc.tile_pool(name="pcol", bufs=3, space="PSUM") as pcol,
            tc.tile_pool(name="pproj", bufs=1, space="PSUM") as pproj,
        ):
            # lhsT for the main pass: columns 0..31 = M^T rows for this
            # n-block, column 32 = ones (plain column sum). fp16, like the
            # streamed S tiles, for full-rate PE; PSUM accumulates fp32.
            lhsT = small.tile([P, NB, G + 1], _F16, tag="lhsT")
            nc.gpsimd.dma_start(out=lhsT[:, :, 0:G],
                                in_=mt_loc.rearrange("(nb p) g -> p nb g", p=P))
            nc.vector.memset(lhsT[:, :, G:G + 1], 1.0)
            lhsT32 = small.tile([P, NB, G], _F32, tag="lhsT32")
            nc.scalar.dma_start(out=lhsT32,
                                in_=mt_loc.rearrange("(nb p) g -> p nb g", p=P))
            mtn = small.tile([P, CT, G], _F32, tag="mtn")
            nc.scalar.dma_start(out=mtn, in_=mt_neg.rearrange("(mc p) g -> p mc g", p=P))
            cg = small.tile([P, 2, NB, T], _F32, tag="cg")
            nc.scalar.dma_start(out=cg[:, 0], in_=cons.rearrange("(nb p) t -> p nb t", p=P))
            nc.scalar.dma_start(out=cg[:, 1], in_=gen.rearrange("(nb p) t -> p nb t", p=P))

            colacc = small.tile([1, N * T], _F32, tag="colacc")
            colT = small.tile([P, CT, T], _F32, tag="colT")
            rowacc = small.tile([G, NEV, T], _F32, tag="rowacc")
            rowsum = small.tile([G, T], _F32, tag="rowsum")
            out_sb = small.tile([G, 3, T], _F32, tag="out_sb")

            pgc = pproj.tile([G, T], _F32, tag="pgc")
            pgg = pproj.tile([G, T], _F32, tag="pgg")
            pcp = pproj.tile([G, T], _F32, tag="pgc")  # reuses pgc's bank (pgc retires early)

            # pgc / pgg: tiny projections of consumption / generation.
            for nb in range(NB):
                nc.tensor.matmul(pgc, lhsT32[:, nb], cg[:, 0, nb],
                                 start=(nb == 0), stop=(nb == NB - 1))
                nc.tensor.matmul(pgg, lhsT32[:, nb], cg[:, 1, nb],
                                 start=(nb == 0), stop=(nb == NB - 1))
            nc.scalar.copy(out=out_sb[:, 0], in_=pgc)
            nc.scalar.copy(out=out_sb[:, 1], in_=pgg)
            nc.scalar.dma_start(out=out[0:2].rearrange("k g t -> g k t"),
                                in_=out_sb[:, 0:2])

            # Stream all of S up front on the SP HWDGE ring.
            s4 = s.rearrange("(nb p) (mb q) t -> mb nb p (q t)", p=P, q=QM)
            stiles = {}
            for mb in range(MB):
                for nb in range(NB):
                    st = spool.tile([P, FREE], _F16, tag="s")
                    nc.gpsimd.dma_start(out=st, in_=s4[mb, nb])
                    stiles[(mb, nb)] = st

            # Flat loop over the 16 evacuation chunks (32 m-columns each).
            # nb outer within each PSUM tile so the stationary weights
            # reload NB times per tile, not per matmul.
            for q in range(NEV):
                pos = q * EV                    # global (m, t) flat offset
                mb, off = divmod(pos, FREE)     # source S tile and offset
                pc = pcol.tile([G + 1, EV], _F32, tag="pc")
                for nb in range(NB):
                    for c0 in range(0, EV, MMCH):
                        cw = min(MMCH, EV - c0)
                        nc.tensor.matmul(
                            pc[:, c0:c0 + cw],
                            lhsT[:, nb],
                            stiles[(mb, nb)][:, off + c0:off + c0 + cw],
                            start=(nb == 0), stop=(nb == NB - 1),
                            skip_group_check=True)
                # Two independent readers drain PSUM concurrently: ACT
                # keeps only the column-sum row, DVE folds the projected
                # rows over this chunk's 32 m-columns.
                nc.scalar.copy(out=colacc[:, pos:pos + EV],
                               in_=pc[G:G + 1, :])
                nc.vector.reduce_sum(
                    out=rowacc[:, q],
                    in_=pc[0:G, :].rearrange("p (m t) -> p t m", t=T),
                    axis=mybir.AxisListType.X,
                )
                # At each 128-m boundary: column sums to m-partitions
                # (ACT HWDGE ring, tiny) and fold -M^T @ colsum in.
                if (q + 1) % (P // EVM) == 0:
                    ct = q // (P // EVM)
                    nc.scalar.dma_start(
                        out=colT[:, ct],
                        in_=colacc[0:1, ct * P * T:(ct + 1) * P * T].rearrange(
                            "p (m t) -> p m t", t=T))
                    nc.tensor.matmul(pcp, mtn[:, ct], colT[:, ct],
                                     start=(ct == 0), stop=(ct == CT - 1))

            nc.vector.reduce_sum(
                out=rowsum, in_=rowacc[:].rearrange("p e t -> p t e"),
                axis=mybir.AxisListType.X)
            nc.vector.tensor_add(out_sb[:, 2], rowsum, pcp)
            nc.scalar.dma_start(out=out[2], in_=out_sb[:, 2])
    nc.compile()
    return nc


_NC_CACHE = None


def _get_program():
    global _NC_CACHE
    if _NC_CACHE is None:
        _NC_CACHE = _build_program()
    return _NC_CACHE


_RUNNER_CACHE = None


def _get_runner():
    """Compiled-once jit(shard_map) executor over 8 cores.

    Mirrors concourse.bass2jax.run_bass_via_pjrt but caches the traced
    function so repeat calls skip retracing/compile-cache lookups."""
    global _RUNNER_CACHE
    if _RUNNER_CACHE is None:
        import jax
        from jax.sharding import Mesh, PartitionSpec
        from jax.experimental.shard_map import shard_map
        from concourse import bass2jax, mybir as mb

        nc = _get_program()
        bass2jax.install_neuronx_cc_hook()
        partition_name = (nc.partition_id_tensor.name
                          if nc.partition_id_tensor else None)
        in_names, out_names, out_avals = [], [], []
        for alloc in nc.m.functions[0].allocations:
            if not isinstance(alloc, mb.MemoryLocationSet):
                continue
            name = alloc.memorylocations[0].name
            if alloc.kind == "ExternalInput":
                if name != partition_name:
                    in_names.append(name)
            elif alloc.kind == "ExternalOutput":
                out_names.append(name)
                out_avals.append(jax.core.ShapedArray(
                    tuple(alloc.tensor_shape), mb.dt.np(alloc.dtype)))
        n_params = len(in_names)
        all_names = in_names + out_names
        if partition_name is not None:
            all_names = all_names + [partition_name]

        def _body(*args):
            operands = list(args)
            if partition_name is not None:
                operands.append(bass2jax.partition_id_tensor())
            outs = bass2jax._bass_exec_p.bind(
                *operands,
                out_avals=tuple(out_avals),
                in_names=tuple(all_names),
                out_names=tuple(out_names),
                lowering_input_output_aliases=(),
                sim_require_finite=True,
                sim_require_nnan=True,
                nc=nc,
            )
            return tuple(outs)

        devices = jax.devices()[:8]
        mesh = Mesh(np.asarray(devices), ("core",))
        n_outs = len(out_names)
        sharded = jax.jit(
            shard_map(_body, mesh=mesh,
                      in_specs=(PartitionSpec("core"),) * (n_params + n_outs),
                      out_specs=(PartitionSpec("core"),) * n_outs,
                      check_rep=False),
            donate_argnums=tuple(range(n_params, n_params + n_outs)),
            keep_unused=True,
        )
        _RUNNER_CACHE = (sharded, in_names[:n_params], out_names, out_avals)
    return _RUNNER_CACHE


def kernel(consumption, generation, sharing_matrix, lv_group_ids,
           valid_lv_mask, imbalance_penalty_weight, _want_results=False,
           **run_kwargs):
    consumption = np.ascontiguousarray(consumption, dtype=np.float32)
    generation = np.ascontiguousarray(generation, dtype=np.float32)
    sharing_matrix = np.ascontiguousarray(sharing_matrix, dtype=np.float32)
    ids = np.asarray(lv_group_ids)
    valid = np.asarray(valid_lv_mask, dtype=np.float32)
    w = np.float32(np.asarray(imbalance_penalty_weight))

    onehot = (ids[None, :] == np.arange(G)[:, None]).astype(np.float32)
    n_unique = np.float32(np.unique(ids).size)
    M = onehot * valid[None, :]                      # [G, N]
    mt = np.ascontiguousarray(M.T)                   # [N, G]
    mt_neg = np.ascontiguousarray(-mt)

    in_maps = []
    for c in range(8):
        b, h = divmod(c, 2)
        sl = slice(h * NLOC, (h + 1) * NLOC)
        in_maps.append({
            "s": np.ascontiguousarray(sharing_matrix[b, sl]),
            "cons": np.ascontiguousarray(consumption[b, sl]),
            "gen": np.ascontiguousarray(generation[b, sl]),
            "mt_loc": np.ascontiguousarray(mt[sl]),
            "mt_neg": mt_neg,
        })
    res = None
    if _want_results or run_kwargs:
        nc = _get_program()
        res = run_bass_kernel_spmd(nc, in_maps, core_ids=list(range(8)),
                                   **run_kwargs)
        parts = np.stack([res.results[c]["out"] for c in range(8)])
    else:
        try:
            fn, in_names, out_names, out_avals = _get_runner()
            concat_in = [np.concatenate([m[name] for m in in_maps], axis=0)
                         for name in in_names]
            zeros = [np.zeros((8 * a.shape[0], *a.shape[1:]), a.dtype)
                     for a in out_avals]
            out_arrs = fn(*concat_in, *zeros)
            parts = np.asarray(out_arrs[out_names.index("out")]).reshape(
                8, 3, G, T)
        except Exception:
            nc = _get_program()
            res = run_bass_kernel_spmd(nc, in_maps, core_ids=list(range(8)))
            parts = np.stack([res.results[c]["out"] for c in range(8)])
    full = parts.reshape(B, 2, 3, G, T).sum(axis=1, dtype=np.float32)
    gc, gg, net = full[:, 0], full[:, 1], full[:, 2]

    imbalance = np.abs(gc - gg + net)
    total = gc + gg + np.float32(EPS)
    pen = np.maximum(imbalance / total - np.float32(TOL), np.float32(0))
    outv = np.float32(pen.sum(dtype=np.float32) * w / n_unique)
    out_arr = np.array(outv, dtype=np.float32)
    if _want_results:
        return out_arr, res
    return out_arr


# revision 39
# speedup vs baseline: 1.0525x; 1.0004x over previous
"""Trainium2 Bass kernel for nn_EnergyBalanceChecker (segment_reduce).

Problem (hardcoded): B=4, N=512, T=24, G=32, TOL=0.05, EPS=1e-6.

  onehot[g,n] = (lv_group_ids[n] == g);  M = onehot * valid_lv_mask
  gc  = einsum('gn,bnt->bgt', M, consumption)
  gg  = einsum('gn,bnt->bgt', M, generation)
  net = einsum('gn,bnt->bgt', M, S.sum(axis=2) - S.sum(axis=1))
  pen = relu(|gc-gg+net| / (gc+gg+eps) - TOL);  out = pen.sum()*w/n_unique

Sharding: 8 cores = 4 batches x 2 halves of the (row) N axis. Each core
reads the contiguous block S[b, h*256:(h+1)*256, :, :] (12.6 MB) once and
emits per-group partials [3, 32, 24] = (pgc, pgg, pnet). The single PE
pass over the data computes both reductions at once: lhsT = [M^T_loc|ones]
gives PSUM rows 0..31 = M-projected rows (still per-(m,t)) and row 32 =
plain column sums. ACT drains PSUM to a [33, N*T] accumulator; DVE then
reduces rows 0..31 over m (row-sum term) while row 32 is regathered to
m-partitions and folded in with -M^T_full weights. Host sums the two
half partials per batch and applies the tiny nonlinear tail (~20 KFLOP).
"""

import sys

import numpy as np

try:
    import concourse  # noqa: F401
except ImportError:
    sys.path.insert(0, "/opt/trn_rl_repo")

import concourse.tile as tile
from concourse import bacc, mybir
from concourse.bass_utils import run_bass_kernel_spmd

B, N, T, G = 4, 512, 24, 32
TOL, EPS = 0.05, 1e-6
P = 128                 # SBUF partitions
NLOC = N // 2           # rows per core (n-half)
NB = NLOC // P          # 2 n-blocks of 128 rows
QM = 64                 # m-columns per streamed S tile
MB = N // QM            # number of (nb-pair) S tiles per core
FREE = QM * T           # free elements per (nb, mb) tile
MMCH = 512              # matmul free-dim chunk
EV = 768                # PSUM evacuation chunk (2 banks, 32 m-columns)
NEV = N * T // EV       # total evacuation chunks (16)
EVM = EV // T           # m-columns per evacuation chunk (32)
CT = N // P             # colT regather chunks of 128 m (4)

_F32 = mybir.dt.float32
_F16 = mybir.dt.float16


def _build_program():
    nc = bacc.Bacc("TRN2", target_bir_lowering=False, debug=False,
                   enable_asserts=False, num_devices=8)
    s = nc.dram_tensor("s", [NLOC, N, T], _F32, kind="ExternalInput").ap()
    cons = nc.dram_tensor("cons", [NLOC, T], _F32, kind="ExternalInput").ap()
    gen = nc.dram_tensor("gen", [NLOC, T], _F32, kind="ExternalInput").ap()
    mt_loc = nc.dram_tensor("mt_loc", [NLOC, G], _F32, kind="ExternalInput").ap()
    mt_neg = nc.dram_tensor("mt_neg", [N, G], _F32, kind="ExternalInput").ap()
    out = nc.dram_tensor("out", [3, G, T], _F32, kind="ExternalOutput").ap()

    with tile.TileContext(nc) as tc:
        with (
            tc.tile_pool(name="spool", bufs=NB * MB) as spool,
            tc.tile_pool(name="small", bufs=1) as small,
            tc.tile_pool(name="pcol", bufs=3, space="PSUM") as pcol,
            tc.tile_pool(name="pproj", bufs=1, space="PSUM") as pproj,
        ):
            # lhsT for the main pass: columns 0..31 = M^T rows for this
            # n-block, column 32 = ones (plain column sum). fp16, like the
            # streamed S tiles, for full-rate PE; PSUM accumulates fp32.
            lhsT = small.tile([P, NB, G + 1], _F16, tag="lhsT")
            nc.gpsimd.dma_start(out=lhsT[:, :, 0:G],
                                in_=mt_loc.rearrange("(nb p) g -> p nb g", p=P))
            nc.vector.memset(lhsT[:, :, G:G + 1], 1.0)
            lhsT32 = small.tile([P, NB, G], _F32, tag="lhsT32")
            nc.scalar.dma_start(out=lhsT32,
                                in_=mt_loc.rearrange("(nb p) g -> p nb g", p=P))
            mtn = small.tile([P, CT, G], _F32, tag="mtn")
            nc.scalar.dma_start(out=mtn, in_=mt_neg.rearrange("(mc p) g -> p mc g", p=P))
            cg = small.tile([P, 2, NB, T], _F32, tag="cg")
            nc.scalar.dma_start(out=cg[:, 0], in_=cons.rearrange("(nb p) t -> p nb t", p=P))
            nc.scalar.dma_start(out=cg[:, 1], in_=gen.rearrange("(nb p) t -> p nb t", p=P))

            colacc = small.tile([1, N * T], _F32, tag="colacc")
            colT = small.tile([P, CT, T], _F32, tag="colT")
            rowacc = small.tile([G, NEV, T], _F32, tag="rowacc")
            rowsum = small.tile([G, T], _F32, tag="rowsum")
            out_sb = small.tile([G, 3, T], _F32, tag="out_sb")

            pgc = pproj.tile([G, T], _F32, tag="pgc")
            pgg = pproj.tile([G, T], _F32, tag="pgg")
            pcp = pproj.tile([G, T], _F32, tag="pgc")  # reuses pgc's bank (pgc retires early)

            # pgc / pgg: tiny projections of consumption / generation.
            for nb in range(NB):
                nc.tensor.matmul(pgc, lhsT32[:, nb], cg[:, 0, nb],
                                 start=(nb == 0), stop=(nb == NB - 1))
                nc.tensor.matmul(pgg, lhsT32[:, nb], cg[:, 1, nb],
                                 start=(nb == 0), stop=(nb == NB - 1))
            nc.scalar.copy(out=out_sb[:, 0], in_=pgc)
            nc.scalar.copy(out=out_sb[:, 1], in_=pgg)
            nc.scalar.dma_start(out=out[0:2].rearrange("k g t -> g k t"),
                                in_=out_sb[:, 0:2])

            # Stream all of S up front on the SP HWDGE ring.
            s4 = s.rearrange("(nb p) (mb q) t -> mb nb p (q t)", p=P, q=QM)
            stiles = {}
            for mb in range(MB):
                for nb in range(NB):
                    st = spool.tile([P, FREE], _F16, tag="s")
                    nc.gpsimd.dma_start(out=st, in_=s4[mb, nb])
                    stiles[(mb, nb)] = st

            # Flat loop over the 16 evacuation chunks (32 m-columns each).
            # nb outer within each PSUM tile so the stationary weights
            # reload NB times per tile, not per matmul.
            for q in range(NEV):
                pos = q * EV                    # global (m, t) flat offset
                mb, off = divmod(pos, FREE)     # source S tile and offset
                pc = pcol.tile([G + 1, EV], _F32, tag="pc")
                for nb in range(NB):
                    for c0 in range(0, EV, MMCH):
                        cw = min(MMCH, EV - c0)
                        nc.tensor.matmul(
                            pc[:, c0:c0 + cw],
                            lhsT[:, nb],
                            stiles[(mb, nb)][:, off + c0:off + c0 + cw],
                            start=(nb == 0), stop=(nb == NB - 1),
                            skip_group_check=True)
                # Two independent readers drain PSUM concurrently: ACT
                # keeps only the column-sum row, DVE folds the projected
                # rows over this chunk's 32 m-columns.
                nc.scalar.copy(out=colacc[:, pos:pos + EV],
                               in_=pc[G:G + 1, :])
                nc.vector.reduce_sum(
                    out=rowacc[:, q],
                    in_=pc[0:G, :].rearrange("p (m t) -> p t m", t=T),
                    axis=mybir.AxisListType.X,
                )
                # At each tile (64-m) boundary: column sums to
                # m-partitions (ACT HWDGE ring, tiny), so the last
                # regather only waits on the final evacuation.
                if (q + 1) % (QM // EVM) == 0:
                    ct = q // (QM // EVM)
                    po = QM * (ct % (P // QM))
                    nc.scalar.dma_start(
                        out=colT[po:po + QM, ct // (P // QM), :],
                        in_=colacc[0:1, ct * QM * T:(ct + 1) * QM * T].rearrange(
                            "p (m t) -> p m t", t=T))

            # Deferred -M^T @ colsum matmuls (K=64 each; PE is in-order,
            # inlining them would stall the chunk stream on colT DMAs).
            for ct in range(MB):
                po = QM * (ct % (P // QM))
                nc.tensor.matmul(pcp, mtn[po:po + QM, ct // (P // QM), :],
                                 colT[po:po + QM, ct // (P // QM), :],
                                 start=(ct == 0), stop=(ct == MB - 1),
                                 skip_group_check=True)

            nc.vector.reduce_sum(
                out=rowsum, in_=rowacc[:].rearrange("p e t -> p t e"),
                axis=mybir.AxisListType.X)
            nc.vector.tensor_add(out_sb[:, 2], rowsum, pcp)
            nc.sync.dma_start(out=out[2], in_=out_sb[:, 2])
    nc.compile()
    return nc


_NC_CACHE = None


def _get_program():
    global _NC_CACHE
    if _NC_CACHE is None:
        _NC_CACHE = _build_program()
    return _NC_CACHE


_RUNNER_CACHE = None


def _get_runner():
    """Compiled-once jit(shard_map) executor over 8 cores.

    Mirrors concourse.bass2jax.run_bass_via_pjrt but caches the traced
    function so repeat calls skip retracing/compile-cache lookups."""
    global _RUNNER_CACHE
    if _RUNNER_CACHE is None:
        import jax
        from jax.sharding import Mesh, PartitionSpec
        from jax.experimental.shard_map import shard_map
        from concourse import bass2jax, mybir as mb

        nc = _get_program()
        bass2jax.install_neuronx_cc_hook()
        partition_name = (nc.partition_id_tensor.name
                          if nc.partition_id_tensor else None)
        in_names, out_names, out_avals = [], [], []
        for alloc in nc.m.functions[0].allocations:
            if not isinstance(alloc, mb.MemoryLocationSet):
                continue
            name = alloc.memorylocations[0].name
            if alloc.kind == "ExternalInput":
                if name != partition_name:
                    in_names.append(name)
            elif alloc.kind == "ExternalOutput":
                out_names.append(name)
                out_avals.append(jax.core.ShapedArray(
                    tuple(alloc.tensor_shape), mb.dt.np(alloc.dtype)))
        n_params = len(in_names)
        all_names = in_names + out_names
        if partition_name is not None:
            all_names = all_names + [partition_name]

        def _body(*args):
            operands = list(args)
            if partition_name is not None:
                operands.append(bass2jax.partition_id_tensor())
            outs = bass2jax._bass_exec_p.bind(
                *operands,
                out_avals=tuple(out_avals),
                in_names=tuple(all_names),
                out_names=tuple(out_names),
                lowering_input_output_aliases=(),
                sim_require_finite=True,
                sim_require_nnan=True,
                nc=nc,
            )
            return tuple(outs)

        devices = jax.devices()[:8]
        mesh = Mesh(np.asarray(devices), ("core",))
        n_outs = len(out_names)
        sharded = jax.jit(
            shard_map(_body, mesh=mesh,
                      in_specs=(PartitionSpec("core"),) * (n_params + n_outs),
                      out_specs=(PartitionSpec("core"),) * n_outs,
                      check_rep=False),
            donate_argnums=tuple(range(n_params, n_params + n_outs)),
            keep_unused=True,
        )
        _RUNNER_CACHE = (sharded, in_names[:n_params], out_names, out_avals)
    return _RUNNER_CACHE


def kernel(consumption, generation, sharing_matrix, lv_group_ids,
           valid_lv_mask, imbalance_penalty_weight, _want_results=False,
           **run_kwargs):
    consumption = np.ascontiguousarray(consumption, dtype=np.float32)
    generation = np.ascontiguousarray(generation, dtype=np.float32)
    sharing_matrix = np.ascontiguousarray(sharing_matrix, dtype=np.float32)
    ids = np.asarray(lv_group_ids)
    valid = np.asarray(valid_lv_mask, dtype=np.float32)
    w = np.float32(np.asarray(imbalance_penalty_weight))

    onehot = (ids[None, :] == np.arange(G)[:, None]).astype(np.float32)
    n_unique = np.float32(np.unique(ids).size)
    M = onehot * valid[None, :]                      # [G, N]
    mt = np.ascontiguousarray(M.T)                   # [N, G]
    mt_neg = np.ascontiguousarray(-mt)

    in_maps = []
    for c in range(8):
        b, h = divmod(c, 2)
        sl = slice(h * NLOC, (h + 1) * NLOC)
        in_maps.append({
            "s": np.ascontiguousarray(sharing_matrix[b, sl]),
            "cons": np.ascontiguousarray(consumption[b, sl]),
            "gen": np.ascontiguousarray(generation[b, sl]),
            "mt_loc": np.ascontiguousarray(mt[sl]),
            "mt_neg": mt_neg,
        })
    res = None
    if _want_results or run_kwargs:
        nc = _get_program()
        res = run_bass_kernel_spmd(nc, in_maps, core_ids=list(range(8)),
                                   **run_kwargs)
        parts = np.stack([res.results[c]["out"] for c in range(8)])
    else:
        try:
            fn, in_names, out_names, out_avals = _get_runner()
            concat_in = [np.concatenate([m[name] for m in in_maps], axis=0)
                         for name in in_names]
            zeros = [np.zeros((8 * a.shape[0], *a.shape[1:]), a.dtype)
                     for a in out_avals]
            out_arrs = fn(*concat_in, *zeros)
            parts = np.asarray(out_arrs[out_names.index("out")]).reshape(
                8, 3, G, T)
        except Exception:
            nc = _get_program()
            res = run_bass_kernel_spmd(nc, in_maps, core_ids=list(range(8)))
            parts = np.stack([res.results[c]["out"] for c in range(8)])
    full = parts.reshape(B, 2, 3, G, T).sum(axis=1, dtype=np.float32)
    gc, gg, net = full[:, 0], full[:, 1], full[:, 2]

    imbalance = np.abs(gc - gg + net)
    total = gc + gg + np.float32(EPS)
    pen = np.maximum(imbalance / total - np.float32(TOL), np.float32(0))
    outv = np.float32(pen.sum(dtype=np.float32) * w / n_unique)
    out_arr = np.array(outv, dtype=np.float32)
    if _want_results:
        return out_arr, res
    return out_arr


# revision 40
# speedup vs baseline: 1.1643x; 1.1062x over previous
"""Trainium2 Bass kernel for nn_EnergyBalanceChecker (segment_reduce).

Problem (hardcoded): B=4, N=512, T=24, G=32, TOL=0.05, EPS=1e-6.

  onehot[g,n] = (lv_group_ids[n] == g);  M = onehot * valid_lv_mask
  gc  = einsum('gn,bnt->bgt', M, consumption)
  gg  = einsum('gn,bnt->bgt', M, generation)
  net = einsum('gn,bnt->bgt', M, S.sum(axis=2) - S.sum(axis=1))
  pen = relu(|gc-gg+net| / (gc+gg+eps) - TOL);  out = pen.sum()*w/n_unique

Sharding: 8 cores = 4 batches x 2 halves of the (row) N axis. Each core
reads the contiguous block S[b, h*256:(h+1)*256, :, :] (12.6 MB) once and
emits per-group partials [3, 32, 24] = (pgc, pgg, pnet). The single PE
pass over the data computes both reductions at once: lhsT = [M^T_loc|ones]
gives PSUM rows 0..31 = M-projected rows (still per-(m,t)) and row 32 =
plain column sums. ACT drains PSUM to a [33, N*T] accumulator; DVE then
reduces rows 0..31 over m (row-sum term) while row 32 is regathered to
m-partitions and folded in with -M^T_full weights. Host sums the two
half partials per batch and applies the tiny nonlinear tail (~20 KFLOP).
"""

import sys

import numpy as np

try:
    import concourse  # noqa: F401
except ImportError:
    sys.path.insert(0, "/opt/trn_rl_repo")

import concourse.tile as tile
from concourse import bacc, mybir
from concourse.bass_utils import run_bass_kernel_spmd

B, N, T, G = 4, 512, 24, 32
TOL, EPS = 0.05, 1e-6
P = 128                 # SBUF partitions
NLOC = N // 2           # rows per core (n-half)
NB = NLOC // P          # 2 n-blocks of 128 rows
QM = 64                 # m-columns per streamed S tile
MB = N // QM            # number of (nb-pair) S tiles per core
FREE = QM * T           # free elements per (nb, mb) tile
MMCH = 512              # matmul free-dim chunk
EV = 768                # PSUM evacuation chunk (2 banks, 32 m-columns)
NEV = N * T // EV       # total evacuation chunks (16)
EVM = EV // T           # m-columns per evacuation chunk (32)
CT = N // P             # colT regather chunks of 128 m (4)

_F32 = mybir.dt.float32
_F16 = mybir.dt.float16


def _build_program():
    nc = bacc.Bacc("TRN2", target_bir_lowering=False, debug=False,
                   enable_asserts=False, num_devices=8)
    s = nc.dram_tensor("s", [NLOC, N, T], _F32, kind="ExternalInput").ap()
    cons = nc.dram_tensor("cons", [NLOC, T], _F32, kind="ExternalInput").ap()
    gen = nc.dram_tensor("gen", [NLOC, T], _F32, kind="ExternalInput").ap()
    mt_loc = nc.dram_tensor("mt_loc", [NLOC, G], _F32, kind="ExternalInput").ap()
    mt_neg = nc.dram_tensor("mt_neg", [N, G], _F32, kind="ExternalInput").ap()
    out = nc.dram_tensor("out", [3, G, T], _F32, kind="ExternalOutput").ap()

    with tile.TileContext(nc) as tc:
        with (
            tc.tile_pool(name="spool", bufs=NB * MB) as spool,
            tc.tile_pool(name="small", bufs=1) as small,
            tc.tile_pool(name="pcol", bufs=3, space="PSUM") as pcol,
            tc.tile_pool(name="pproj", bufs=1, space="PSUM") as pproj,
        ):
            # lhsT for the main pass: columns 0..31 = M^T rows for this
            # n-block, column 32 = ones (plain column sum). fp16, like the
            # streamed S tiles, for full-rate PE; PSUM accumulates fp32.
            lhsT32 = small.tile([P, NB, G], _F32, tag="lhsT32")
            nc.scalar.dma_start(out=lhsT32,
                                in_=mt_loc.rearrange("(nb p) g -> p nb g", p=P))
            # Cast on DVE (32 cycles) rather than a SWDGE cast-DMA: the
            # Q7's descriptor emission must stay free for the S stream.
            lhsT = small.tile([P, NB, G + 1], _F16, tag="lhsT")
            nc.vector.tensor_copy(out=lhsT[:, :, 0:G], in_=lhsT32)
            nc.vector.memset(lhsT[:, :, G:G + 1], 1.0)
            mtn = small.tile([P, CT, G], _F32, tag="mtn")
            nc.scalar.dma_start(out=mtn, in_=mt_neg.rearrange("(mc p) g -> p mc g", p=P))
            cg = small.tile([P, 2, NB, T], _F32, tag="cg")
            nc.scalar.dma_start(out=cg[:, 0], in_=cons.rearrange("(nb p) t -> p nb t", p=P))
            nc.scalar.dma_start(out=cg[:, 1], in_=gen.rearrange("(nb p) t -> p nb t", p=P))

            colacc = small.tile([1, N * T], _F32, tag="colacc")
            colT = small.tile([P, CT, T], _F32, tag="colT")
            rowacc = small.tile([G, NEV, T], _F32, tag="rowacc")
            rowsum = small.tile([G, T], _F32, tag="rowsum")
            out_sb = small.tile([G, 3, T], _F32, tag="out_sb")

            pgc = pproj.tile([G, T], _F32, tag="pgc")
            pgg = pproj.tile([G, T], _F32, tag="pgg")
            pcp = pproj.tile([G, T], _F32, tag="pgc")  # reuses pgc's bank (pgc retires early)

            # pgc / pgg: tiny projections of consumption / generation.
            for nb in range(NB):
                nc.tensor.matmul(pgc, lhsT32[:, nb], cg[:, 0, nb],
                                 start=(nb == 0), stop=(nb == NB - 1))
                nc.tensor.matmul(pgg, lhsT32[:, nb], cg[:, 1, nb],
                                 start=(nb == 0), stop=(nb == NB - 1))
            nc.scalar.copy(out=out_sb[:, 0], in_=pgc)
            nc.scalar.copy(out=out_sb[:, 1], in_=pgg)
            nc.scalar.dma_start(out=out[0:2].rearrange("k g t -> g k t"),
                                in_=out_sb[:, 0:2])

            # Stream all of S up front on the SP HWDGE ring.
            s4 = s.rearrange("(nb p) (mb q) t -> mb nb p (q t)", p=P, q=QM)
            stiles = {}
            for mb in range(MB):
                for nb in range(NB):
                    st = spool.tile([P, FREE], _F16, tag="s")
                    nc.gpsimd.dma_start(out=st, in_=s4[mb, nb])
                    stiles[(mb, nb)] = st

            # Flat loop over the 16 evacuation chunks (32 m-columns each).
            # nb outer within each PSUM tile so the stationary weights
            # reload NB times per tile, not per matmul.
            for q in range(NEV):
                pos = q * EV                    # global (m, t) flat offset
                mb, off = divmod(pos, FREE)     # source S tile and offset
                pc = pcol.tile([G + 1, EV], _F32, tag="pc")
                for nb in range(NB):
                    for c0 in range(0, EV, MMCH):
                        cw = min(MMCH, EV - c0)
                        nc.tensor.matmul(
                            pc[:, c0:c0 + cw],
                            lhsT[:, nb],
                            stiles[(mb, nb)][:, off + c0:off + c0 + cw],
                            start=(nb == 0), stop=(nb == NB - 1),
                            skip_group_check=True)
                # Two independent readers drain PSUM concurrently: ACT
                # keeps only the column-sum row, DVE folds the projected
                # rows over this chunk's 32 m-columns.
                nc.scalar.copy(out=colacc[:, pos:pos + EV],
                               in_=pc[G:G + 1, :])
                nc.vector.reduce_sum(
                    out=rowacc[:, q],
                    in_=pc[0:G, :].rearrange("p (m t) -> p t m", t=T),
                    axis=mybir.AxisListType.X,
                )
                # At each tile (64-m) boundary: column sums to
                # m-partitions (ACT HWDGE ring, tiny), so the last
                # regather only waits on the final evacuation.
                if (q + 1) % (QM // EVM) == 0:
                    ct = q // (QM // EVM)
                    po = QM * (ct % (P // QM))
                    nc.scalar.dma_start(
                        out=colT[po:po + QM, ct // (P // QM), :],
                        in_=colacc[0:1, ct * QM * T:(ct + 1) * QM * T].rearrange(
                            "p (m t) -> p m t", t=T))

            # Deferred -M^T @ colsum matmuls (K=64 each; PE is in-order,
            # inlining them would stall the chunk stream on colT DMAs).
            for ct in range(MB):
                po = QM * (ct % (P // QM))
                nc.tensor.matmul(pcp, mtn[po:po + QM, ct // (P // QM), :],
                                 colT[po:po + QM, ct // (P // QM), :],
                                 start=(ct == 0), stop=(ct == MB - 1),
                                 skip_group_check=True)

            nc.vector.reduce_sum(
                out=rowsum, in_=rowacc[:].rearrange("p e t -> p t e"),
                axis=mybir.AxisListType.X)
            nc.vector.tensor_add(out_sb[:, 2], rowsum, pcp)
            nc.sync.dma_start(out=out[2], in_=out_sb[:, 2])
    nc.compile()
    return nc


_NC_CACHE = None


def _get_program():
    global _NC_CACHE
    if _NC_CACHE is None:
        _NC_CACHE = _build_program()
    return _NC_CACHE


_RUNNER_CACHE = None


def _get_runner():
    """Compiled-once jit(shard_map) executor over 8 cores.

    Mirrors concourse.bass2jax.run_bass_via_pjrt but caches the traced
    function so repeat calls skip retracing/compile-cache lookups."""
    global _RUNNER_CACHE
    if _RUNNER_CACHE is None:
        import jax
        from jax.sharding import Mesh, PartitionSpec
        from jax.experimental.shard_map import shard_map
        from concourse import bass2jax, mybir as mb

        nc = _get_program()
        bass2jax.install_neuronx_cc_hook()
        partition_name = (nc.partition_id_tensor.name
                          if nc.partition_id_tensor else None)
        in_names, out_names, out_avals = [], [], []
        for alloc in nc.m.functions[0].allocations:
            if not isinstance(alloc, mb.MemoryLocationSet):
                continue
            name = alloc.memorylocations[0].name
            if alloc.kind == "ExternalInput":
                if name != partition_name:
                    in_names.append(name)
            elif alloc.kind == "ExternalOutput":
                out_names.append(name)
                out_avals.append(jax.core.ShapedArray(
                    tuple(alloc.tensor_shape), mb.dt.np(alloc.dtype)))
        n_params = len(in_names)
        all_names = in_names + out_names
        if partition_name is not None:
            all_names = all_names + [partition_name]

        def _body(*args):
            operands = list(args)
            if partition_name is not None:
                operands.append(bass2jax.partition_id_tensor())
            outs = bass2jax._bass_exec_p.bind(
                *operands,
                out_avals=tuple(out_avals),
                in_names=tuple(all_names),
                out_names=tuple(out_names),
                lowering_input_output_aliases=(),
                sim_require_finite=True,
                sim_require_nnan=True,
                nc=nc,
            )
            return tuple(outs)

        devices = jax.devices()[:8]
        mesh = Mesh(np.asarray(devices), ("core",))
        n_outs = len(out_names)
        sharded = jax.jit(
            shard_map(_body, mesh=mesh,
                      in_specs=(PartitionSpec("core"),) * (n_params + n_outs),
                      out_specs=(PartitionSpec("core"),) * n_outs,
                      check_rep=False),
            donate_argnums=tuple(range(n_params, n_params + n_outs)),
            keep_unused=True,
        )
        _RUNNER_CACHE = (sharded, in_names[:n_params], out_names, out_avals)
    return _RUNNER_CACHE


def kernel(consumption, generation, sharing_matrix, lv_group_ids,
           valid_lv_mask, imbalance_penalty_weight, _want_results=False,
           **run_kwargs):
    consumption = np.ascontiguousarray(consumption, dtype=np.float32)
    generation = np.ascontiguousarray(generation, dtype=np.float32)
    sharing_matrix = np.ascontiguousarray(sharing_matrix, dtype=np.float32)
    ids = np.asarray(lv_group_ids)
    valid = np.asarray(valid_lv_mask, dtype=np.float32)
    w = np.float32(np.asarray(imbalance_penalty_weight))

    onehot = (ids[None, :] == np.arange(G)[:, None]).astype(np.float32)
    n_unique = np.float32(np.unique(ids).size)
    M = onehot * valid[None, :]                      # [G, N]
    mt = np.ascontiguousarray(M.T)                   # [N, G]
    mt_neg = np.ascontiguousarray(-mt)

    in_maps = []
    for c in range(8):
        b, h = divmod(c, 2)
        sl = slice(h * NLOC, (h + 1) * NLOC)
        in_maps.append({
            "s": np.ascontiguousarray(sharing_matrix[b, sl]),
            "cons": np.ascontiguousarray(consumption[b, sl]),
            "gen": np.ascontiguousarray(generation[b, sl]),
            "mt_loc": np.ascontiguousarray(mt[sl]),
            "mt_neg": mt_neg,
        })
    res = None
    if _want_results or run_kwargs:
        nc = _get_program()
        res = run_bass_kernel_spmd(nc, in_maps, core_ids=list(range(8)),
                                   **run_kwargs)
        parts = np.stack([res.results[c]["out"] for c in range(8)])
    else:
        try:
            fn, in_names, out_names, out_avals = _get_runner()
            concat_in = [np.concatenate([m[name] for m in in_maps], axis=0)
                         for name in in_names]
            zeros = [np.zeros((8 * a.shape[0], *a.shape[1:]), a.dtype)
                     for a in out_avals]
            out_arrs = fn(*concat_in, *zeros)
            parts = np.asarray(out_arrs[out_names.index("out")]).reshape(
                8, 3, G, T)
        except Exception:
            nc = _get_program()
            res = run_bass_kernel_spmd(nc, in_maps, core_ids=list(range(8)))
            parts = np.stack([res.results[c]["out"] for c in range(8)])
    full = parts.reshape(B, 2, 3, G, T).sum(axis=1, dtype=np.float32)
    gc, gg, net = full[:, 0], full[:, 1], full[:, 2]

    imbalance = np.abs(gc - gg + net)
    total = gc + gg + np.float32(EPS)
    pen = np.maximum(imbalance / total - np.float32(TOL), np.float32(0))
    outv = np.float32(pen.sum(dtype=np.float32) * w / n_unique)
    out_arr = np.array(outv, dtype=np.float32)
    if _want_results:
        return out_arr, res
    return out_arr
